# revision 6
# baseline (speedup 1.0000x reference)
"""Trainium2 Bass kernel for modulated conv1d (StyleGAN-style Conv1DMod).

Reference computation (per batch sample b):
  wm[k,c,f]  = kern[k,c,f] * coef * (style[b,c] + 1)        (modulate)
  denom[f]   = rsqrt(sum_{k,c} wm[k,c,f]^2)                 (demodulate)
  out[b,f,w] = denom[f] * sum_{k,c} wm[k,c,f] * feat[b,c,w+k-1]   (SAME conv)

Sharding: data-parallel over batch B=8 -> one sample per NeuronCore.
Demodulation is a per-(b,f) linear scale, so it is applied to the conv
*output* tiles (whose partition dim is f) instead of rescaling weights.

v5 structure:
  - contraction rounds use the partition mapping c = 2p + h (h in {0,1})
    instead of c = h*128 + p. Adjacent channel rows then sit on the same
    partition, so kern loads as [128, 2, 256] with one contiguous 2KB run
    per partition (~1us per k-slice vs ~3us for the 1KB-descriptor
    layout) and style loads as [128, 2] with one 8B run per partition
    (no transpose scatter needed). Features just use a
    partition-stride-of-2-rows access pattern (same DMA cost).
  - the conv runs in bf16 (fp32 PSUM accumulate): same 1 col/cycle PE
    rate as fp32r, no "producer must round" verifier constraint, FWL
  - ring assignment: h0 feature chunks on the sync HWDGE ring, h1 on the
    gpsimd SWDGE ring, style+kern+output stores on the scalar HWDGE ring.
    All fp32->bf16 feature casts run on the vector engine.
  - a block of dummy bf16 matmuls at the head keeps the PE busy during
    the initial DMA wait so the HAM clock-gate opens (K=8/8) before the
    real matmul stream starts
  - matmul order: first group bank-major (tracks the piecewise chunk-0
    loads), later groups weight-major (one LDWEIGHTS per 4 matmuls)
  - conv output is demodulated into bf16 staging tiles and stored as
    bf16 (halves store traffic + kernel tail); host upcasts to fp32
"""

import numpy as np

import concourse.bass as bass
import concourse.mybir as mybir
import concourse.tile as tile

B, C, W, K, F = 8, 256, 8192, 3, 256
COEF = 1.0 / float(np.sqrt(K * C))

P = 128
NH = C // P  # 2 contraction rounds per k-tap (c = 2p + h)
FT = F // P  # 2 output-partition tiles
WCHUNK = 2048  # feature chunk width
NJ = W // WCHUNK  # 4 chunks
WTILE = 512  # matmul moving-operand width (psum bank limit)
NI = WCHUNK // WTILE  # 4 w-tiles per chunk
XCOLS = WCHUNK + 2  # chunk + 1-col halo each side

N_WARM = 10  # dummy PE-warmup matmuls (N=256 each)

MAX_WAITS = 1  # walrus codegen in this container rejects >1 sync wait per inst


def _split_sync_waits(nc, limit=MAX_WAITS):
    """Move excess sem-waits onto NoOps inserted before the offending
    instruction (same engine, program order preserved)."""
    uid = 0
    for fn in nc.m.functions:
        for bb in fn.blocks:
            insts = bb.instructions
            changed = False
            newlist = []
            for ins in insts:
                si = ins.sync_info
                if si is not None and len(si.on_wait) > limit:
                    waits = list(si.on_wait)
                    keep = waits[-limit:]
                    excess = waits[:-limit]
                    for k in range(0, len(excess), limit):
                        nop = mybir.InstNoOp(name=f"waitsplit-{uid}", ins=[], outs=[])
                        uid += 1
                        nop.engine = ins.engine
                        nop.sync_info = mybir.SyncInfo(
                            on_wait=excess[k : k + limit], on_update=[]
                        )
                        newlist.append(nop)
                    ins.sync_info = mybir.SyncInfo(
                        on_wait=keep, on_update=list(si.on_update)
                    )
                    changed = True
                newlist.append(ins)
            if changed:
                bb.instructions = newlist


def _conv1dmod_body(tc, feat, style, kern, out):
    nc = tc.nc
    f32 = mybir.dt.float32
    bf16 = mybir.dt.bfloat16

    # feature rows for round h: c = 2p + h  -> [NH, 128, W]
    fview = feat.rearrange("(p h) w -> h p w", h=NH)
    # kern k-slice for round h: [K, 128, NH, F], 2KB contiguous per partition
    kvw = kern.rearrange("k (p h) f -> k p h f", h=NH)

    with (
        tc.tile_pool(name="xbuf", bufs=1) as xbuf,
        tc.tile_pool(name="xraw", bufs=2) as xraw_pool,
        tc.tile_pool(name="wbuf", bufs=1) as wbuf,
        tc.tile_pool(name="stage", bufs=3) as stage_pool,
        tc.tile_pool(name="psum", bufs=7, space="PSUM") as psum_pool,
        tc.tile_pool(name="dpsum", bufs=1, space="PSUM") as dpsum_pool,
    ):
        # ---- PE warmup: dense dummy matmuls while the first DMAs fly.
        # The HAM clock gate needs ~3.4us of sustained PE activity to open
        # to K=8/8; without this the first ~20 real matmuls run at 1.2 GHz.
        wz = wbuf.tile([P, 256], bf16, tag="warmz")
        nc.vector.memset(wz[:], 0.0)
        wps = dpsum_pool.tile([P, 256], f32, tag="dpsum")
        for _ in range(N_WARM):
            nc.tensor.matmul(wps[:], wz[:, :P], wz[:], start=True, stop=True)

        # ---- style + kern on the scalar HWDGE ring (stores come later) ----
        ssty = wbuf.tile([P, NH], f32, tag="ssty")
        nc.scalar.dma_start(ssty[:], style.rearrange("(p h) -> p h", h=NH))
        ksb = [
            wbuf.tile([P, NH, F], f32, tag=f"ksb_{k}", name=f"ksb_{k}")
            for k in range(K)
        ]
        for k in range(K):
            nc.scalar.dma_start(ksb[k][:], kvw[k])

        # ---- feature tiles: fp32 DMA (h0 -> sync ring, h1 -> gpsimd ring)
        # + vector-engine cast to bf16.
        xt = [[None] * NJ for _ in range(NH)]
        dma_eng = [nc.sync, nc.gpsimd]

        def emit_load(j, npieces=1, order=None):
            plan = []
            for h in range(NH):
                t = xbuf.tile([P, XCOLS], bf16, tag=f"x_{h}_{j}", name=f"x_{h}_{j}")
                xt[h][j] = t
                raw = xraw_pool.tile(
                    [P, XCOLS], f32, tag=f"xraw_{h}", name=f"xraw_{h}_{j}"
                )
                lo = j * WCHUNK - 1
                hi = j * WCHUNK + WCHUNK + 1
                dst_lo = 0
                if lo < 0:
                    nc.vector.memset(t[:, 0:1], 0.0)
                    dst_lo = 1
                    lo = 0
                if hi > W:
                    nc.vector.memset(t[:, XCOLS - 1 : XCOLS], 0.0)
                    hi = W
                bounds = np.linspace(lo, hi, npieces + 1).astype(int)
                for pi, (p0, p1) in enumerate(zip(bounds[:-1], bounds[1:])):
                    plan.append((h, t, raw, int(p0), int(p1), dst_lo + int(p0 - lo), pi))
            if order == "piece":
                plan.sort(key=lambda e: (e[6], e[0]))
            for h, t, raw, p0, p1, off, _ in plan:
                ncols = p1 - p0
                dma_eng[h].dma_start(raw[:, off : off + ncols], fview[h, :, p0:p1])
                nc.vector.tensor_copy(t[:, off : off + ncols], raw[:, off : off + ncols])

        emit_load(0, npieces=4, order="piece")
        emit_load(1)

        # ---- modulate weights (bf16 out): wm[k][p,h,f] = ksb*coef*(1+s) ----
        s1 = wbuf.tile([P, NH], f32, tag="s1")
        nc.vector.tensor_scalar(
            s1[:], ssty[:], 1.0, COEF, mybir.AluOpType.add, mybir.AluOpType.mult
        )
        wm = []
        for k in range(K):
            wmt = wbuf.tile([P, NH, F], bf16, tag=f"wm_{k}", name=f"wm_{k}")
            for h in range(NH):
                nc.vector.tensor_scalar_mul(
                    wmt[:, h, :], ksb[k][:, h, :], s1[:, h : h + 1]
                )
            wm.append(wmt)

        def emit_mms(j, ft, bank_major=False):
            """NI psum accumulation groups for (chunk j, ft). Weight-major
            (one (k,h) stationary load feeds all NI moving tiles) unless
            bank_major (first group: tracks piecewise chunk-0 arrival)."""
            pss = [
                psum_pool.tile([P, WTILE], f32, tag="psum", name=f"ps_{j}_{ft}_{i}")
                for i in range(NI)
            ]
            rounds = [(k, h) for k in range(K) for h in range(NH)]
            if bank_major:
                seq = [(i, kh) for i in range(NI) for kh in rounds]
            else:
                seq = [(i, kh) for kh in rounds for i in range(NI)]
            for i, (k, h) in seq:
                nc.tensor.matmul(
                    pss[i][:],
                    wm[k][:, h, ft * P : (ft + 1) * P],
                    xt[h][j][:, i * WTILE + k : i * WTILE + k + WTILE],
                    start=(k == 0 and h == 0),
                    stop=(k == K - 1 and h == NH - 1),
                    skip_group_check=True,
                )
            return pss

        def emit_copies(j, ft, pss):
            """Demodulating PSUM->SBUF bf16 copies + bf16 output stores on
            the scalar HWDGE ring (disjoint from the feature-load rings)."""
            st = stage_pool.tile([P, WCHUNK], bf16, tag="stage")
            for i, ps in enumerate(pss):
                nc.vector.tensor_scalar_mul(
                    st[:, i * WTILE : (i + 1) * WTILE], ps[:], denom[:, ft : ft + 1]
                )
            out_rows = slice(ft * P, (ft + 1) * P)
            # finer stores on the last chunk shorten the end-of-kernel tail
            npieces = 4 if j == NJ - 1 else 2
            piece = WCHUNK // npieces
            for h in range(npieces):
                out_cols = slice(j * WCHUNK + h * piece, j * WCHUNK + (h + 1) * piece)
                nc.scalar.dma_start(
                    out[out_rows, out_cols], st[:, h * piece : (h + 1) * piece]
                )

        # chunk-0 first matmul block goes ahead of everything else
        pss00 = emit_mms(0, 0, bank_major=True)

        # ---- demodulation scale: denom[f] = rsqrt(sum_{k,c} wm^2) ----
        # Emitted after the first conv block so the tiny demod matmuls do
        # not sit at the head of the in-order PE queue waiting on the DVE
        # square/sum chain.
        ssq = []
        for h in range(NH):
            sqs = []
            for k in range(K):
                sqt = wbuf.tile([P, F], f32, tag=f"sq_{h}_{k}", name=f"sq_{h}_{k}")
                nc.vector.tensor_mul(sqt[:], wm[k][:, h, :], wm[k][:, h, :])
                sqs.append(sqt)
            sst = wbuf.tile([P, F], f32, tag=f"ssq_{h}", name=f"ssq_{h}")
            nc.vector.tensor_add(sst[:], sqs[0][:], sqs[1][:])
            nc.vector.tensor_add(sst[:], sst[:], sqs[2][:])
            ssq.append(sst)
        ones = wbuf.tile([P, 1], f32, tag="ones")
        nc.vector.memset(ones[:], 1.0)
        dp = dpsum_pool.tile([P, FT], f32, tag="dpsum")
        for ft in range(FT):
            for h in range(NH):
                nc.tensor.matmul(
                    dp[:, ft : ft + 1],
                    ssq[h][:, ft * P : (ft + 1) * P],
                    ones[:],
                    start=(h == 0),
                    stop=(h == NH - 1),
                )
        denom = wbuf.tile([P, FT], f32, tag="denom")
        nc.scalar.activation(denom[:], dp[:], mybir.ActivationFunctionType.Sqrt)
        nc.vector.reciprocal(denom[:], denom[:])

        # ---- conv: chunk loads stay one chunk ahead of the matmul stream ----
        emit_copies(0, 0, pss00)
        emit_copies(0, 1, emit_mms(0, 1))
        for j in range(1, NJ):
            if j + 1 < NJ:
                emit_load(j + 1)
            for ft in range(FT):
                emit_copies(j, ft, emit_mms(j, ft))


def build_bass():
    nc = bass.Bass(name="conv1dmod")
    feat = nc.dram_tensor("feature", [C, W], mybir.dt.float32, kind="ExternalInput")
    style = nc.dram_tensor("style", [C], mybir.dt.float32, kind="ExternalInput")
    kern = nc.dram_tensor("kern", [K, C, F], mybir.dt.float32, kind="ExternalInput")
    out = nc.dram_tensor("out", [F, W], mybir.dt.bfloat16, kind="ExternalOutput")
    with tile.TileContext(nc) as tc:
        _conv1dmod_body(tc, feat, style, kern, out)
    _split_sync_waits(nc)
    return nc


_NC_CACHE = None


def kernel(feature, style, kernel):
    """Full-input entry point: shard over batch across 8 cores, run, gather."""
    global _NC_CACHE
    from concourse.bass_utils import run_bass_kernel_spmd

    if _NC_CACHE is None:
        _NC_CACHE = build_bass()
    nc = _NC_CACHE

    feature = np.ascontiguousarray(feature, dtype=np.float32)
    style = np.ascontiguousarray(style, dtype=np.float32)
    kernel = np.ascontiguousarray(kernel, dtype=np.float32)

    in_maps = [
        {"feature": feature[b], "style": style[b], "kern": kernel} for b in range(B)
    ]
    res = run_bass_kernel_spmd(nc, in_maps, core_ids=list(range(B)))
    return np.stack(
        [np.asarray(r["out"]).astype(np.float32) for r in res.results], axis=0
    )


# revision 9
# speedup vs baseline: 1.0776x; 1.0776x over previous
"""Trainium2 Bass kernel for modulated conv1d (StyleGAN-style Conv1DMod).

Reference computation (per batch sample b):
  wm[k,c,f]  = kern[k,c,f] * coef * (style[b,c] + 1)        (modulate)
  denom[f]   = rsqrt(sum_{k,c} wm[k,c,f]^2)                 (demodulate)
  out[b,f,w] = denom[f] * sum_{k,c} wm[k,c,f] * feat[b,c,w+k-1]   (SAME conv)

Sharding: data-parallel over batch B=8 -> one sample per NeuronCore.
Demodulation is a per-(b,f) linear scale, so it is applied to the conv
*output* tiles (whose partition dim is f) instead of rescaling weights.

v5 structure:
  - contraction rounds use the partition mapping c = 2p + h (h in {0,1})
    instead of c = h*128 + p. Adjacent channel rows then sit on the same
    partition, so kern loads as [128, 2, 256] with one contiguous 2KB run
    per partition (~1us per k-slice vs ~3us for the 1KB-descriptor
    layout) and style loads as [128, 2] with one 8B run per partition
    (no transpose scatter needed). Features just use a
    partition-stride-of-2-rows access pattern (same DMA cost).
  - the conv runs in bf16 (fp32 PSUM accumulate): same 1 col/cycle PE
    rate as fp32r, no "producer must round" verifier constraint, FWL
  - ring assignment: h0 feature chunks on the sync HWDGE ring, h1 on the
    gpsimd SWDGE ring, style+kern+output stores on the scalar HWDGE ring.
    All fp32->bf16 feature casts run on the vector engine.
  - a block of dummy bf16 matmuls at the head keeps the PE busy during
    the initial DMA wait so the HAM clock-gate opens (K=8/8) before the
    real matmul stream starts
  - matmul order: first group bank-major (tracks the piecewise chunk-0
    loads), later groups weight-major (one LDWEIGHTS per 4 matmuls)
  - conv output is demodulated into bf16 staging tiles and stored as
    bf16 (halves store traffic + kernel tail); host upcasts to fp32
"""

import numpy as np

import concourse.bass as bass
import concourse.mybir as mybir
import concourse.tile as tile

B, C, W, K, F = 8, 256, 8192, 3, 256
COEF = 1.0 / float(np.sqrt(K * C))

P = 128
NH = C // P  # 2 contraction rounds per k-tap (c = 2p + h)
FT = F // P  # 2 output-partition tiles
WCHUNK = 2048  # feature chunk width
NJ = W // WCHUNK  # 4 chunks
WTILE = 512  # matmul moving-operand width (psum bank limit)
NI = WCHUNK // WTILE  # 4 w-tiles per chunk
XCOLS = WCHUNK + 2  # chunk + 1-col halo each side

N_WARM = 11  # dummy PE-warmup matmuls (N=256 each)

MAX_WAITS = 1  # walrus codegen in this container rejects >1 sync wait per inst


def _split_sync_waits(nc, limit=MAX_WAITS):
    """Move excess sem-waits onto NoOps inserted before the offending
    instruction (same engine, program order preserved)."""
    uid = 0
    for fn in nc.m.functions:
        for bb in fn.blocks:
            insts = bb.instructions
            changed = False
            newlist = []
            for ins in insts:
                si = ins.sync_info
                if si is not None and len(si.on_wait) > limit:
                    waits = list(si.on_wait)
                    keep = waits[-limit:]
                    excess = waits[:-limit]
                    for k in range(0, len(excess), limit):
                        nop = mybir.InstNoOp(name=f"waitsplit-{uid}", ins=[], outs=[])
                        uid += 1
                        nop.engine = ins.engine
                        nop.sync_info = mybir.SyncInfo(
                            on_wait=excess[k : k + limit], on_update=[]
                        )
                        newlist.append(nop)
                    ins.sync_info = mybir.SyncInfo(
                        on_wait=keep, on_update=list(si.on_update)
                    )
                    changed = True
                newlist.append(ins)
            if changed:
                bb.instructions = newlist


def _conv1dmod_body(tc, feat, style, kern, out):
    nc = tc.nc
    f32 = mybir.dt.float32
    bf16 = mybir.dt.bfloat16

    # feature rows for round h: c = 2p + h  -> [NH, 128, W]
    fview = feat.rearrange("(p h) w -> h p w", h=NH)
    # kern k-slice for round h: [K, 128, NH, F], 2KB contiguous per partition
    kvw = kern.rearrange("k (p h) f -> k p h f", h=NH)

    with (
        tc.tile_pool(name="xbuf", bufs=1) as xbuf,
        tc.tile_pool(name="xraw", bufs=2) as xraw_pool,
        tc.tile_pool(name="wbuf", bufs=1) as wbuf,
        tc.tile_pool(name="stage", bufs=3) as stage_pool,
        tc.tile_pool(name="psum", bufs=7, space="PSUM") as psum_pool,
        tc.tile_pool(name="dpsum", bufs=1, space="PSUM") as dpsum_pool,
    ):
        # ---- PE warmup: dense dummy matmuls while the first DMAs fly.
        # The HAM clock gate needs ~3.4us of sustained PE activity to open
        # to K=8/8; without this the first ~20 real matmuls run at 1.2 GHz.
        wz = wbuf.tile([P, 256], bf16, tag="warmz")
        nc.vector.memset(wz[:], 0.0)
        wps = dpsum_pool.tile([P, 256], f32, tag="dpsum")
        for _ in range(N_WARM):
            nc.tensor.matmul(wps[:], wz[:, :P], wz[:], start=True, stop=True)

        # ---- style + kern on the scalar HWDGE ring (stores come later) ----
        ssty = wbuf.tile([P, NH], f32, tag="ssty")
        nc.scalar.dma_start(ssty[:], style.rearrange("(p h) -> p h", h=NH))
        ksb = [
            wbuf.tile([P, NH, F], f32, tag=f"ksb_{k}", name=f"ksb_{k}")
            for k in range(K)
        ]
        for k in range(K):
            nc.scalar.dma_start(ksb[k][:], kvw[k])

        # ---- feature tiles: fp32 DMA (h0 -> sync ring, h1 -> gpsimd ring)
        # + cast to bf16 on the gpsimd (h0) / scalar (h1) engines. The DVE
        # must stay free for the PSUM demod copies (the v5 trace showed an
        # all-casts-on-DVE build starving the matmul stream).
        xt = [[None] * NJ for _ in range(NH)]
        dma_eng = [nc.sync, nc.gpsimd]
        cvt_eng = [nc.gpsimd.tensor_copy, nc.scalar.copy]

        def emit_load(j, npieces=1, order=None):
            plan = []
            for h in range(NH):
                t = xbuf.tile([P, XCOLS], bf16, tag=f"x_{h}_{j}", name=f"x_{h}_{j}")
                xt[h][j] = t
                raw = xraw_pool.tile(
                    [P, XCOLS], f32, tag=f"xraw_{h}", name=f"xraw_{h}_{j}"
                )
                lo = j * WCHUNK - 1
                hi = j * WCHUNK + WCHUNK + 1
                dst_lo = 0
                if lo < 0:
                    nc.vector.memset(t[:, 0:1], 0.0)
                    dst_lo = 1
                    lo = 0
                if hi > W:
                    nc.vector.memset(t[:, XCOLS - 1 : XCOLS], 0.0)
                    hi = W
                bounds = np.linspace(lo, hi, npieces + 1).astype(int)
                for pi, (p0, p1) in enumerate(zip(bounds[:-1], bounds[1:])):
                    plan.append((h, t, raw, int(p0), int(p1), dst_lo + int(p0 - lo), pi))
            if order == "piece":
                plan.sort(key=lambda e: (e[6], e[0]))
            for h, t, raw, p0, p1, off, _ in plan:
                ncols = p1 - p0
                dma_eng[h].dma_start(raw[:, off : off + ncols], fview[h, :, p0:p1])
                cvt_eng[h](t[:, off : off + ncols], raw[:, off : off + ncols])

        emit_load(0, npieces=4, order="piece")
        emit_load(1)

        # ---- modulate weights (bf16 out): wm[k][p,h,f] = ksb*coef*(1+s) ----
        s1 = wbuf.tile([P, NH], f32, tag="s1")
        nc.vector.tensor_scalar(
            s1[:], ssty[:], 1.0, COEF, mybir.AluOpType.add, mybir.AluOpType.mult
        )
        wm = []
        for k in range(K):
            wmt = wbuf.tile([P, NH, F], bf16, tag=f"wm_{k}", name=f"wm_{k}")
            for h in range(NH):
                nc.vector.tensor_scalar_mul(
                    wmt[:, h, :], ksb[k][:, h, :], s1[:, h : h + 1]
                )
            wm.append(wmt)

        def emit_mms(j, ft, bank_major=False):
            """NI psum accumulation groups for (chunk j, ft). Weight-major
            (one (k,h) stationary load feeds all NI moving tiles) unless
            bank_major (first group: tracks piecewise chunk-0 arrival)."""
            pss = [
                psum_pool.tile([P, WTILE], f32, tag="psum", name=f"ps_{j}_{ft}_{i}")
                for i in range(NI)
            ]
            rounds = [(k, h) for k in range(K) for h in range(NH)]
            if bank_major:
                seq = [(i, kh) for i in range(NI) for kh in rounds]
            else:
                seq = [(i, kh) for kh in rounds for i in range(NI)]
            for i, (k, h) in seq:
                nc.tensor.matmul(
                    pss[i][:],
                    wm[k][:, h, ft * P : (ft + 1) * P],
                    xt[h][j][:, i * WTILE + k : i * WTILE + k + WTILE],
                    start=(k == 0 and h == 0),
                    stop=(k == K - 1 and h == NH - 1),
                    skip_group_check=True,
                )
            return pss

        def emit_copies(j, ft, pss):
            """Demodulating PSUM->SBUF bf16 copies + bf16 output stores on
            the scalar HWDGE ring (disjoint from the feature-load rings)."""
            st = stage_pool.tile([P, WCHUNK], bf16, tag="stage")
            for i, ps in enumerate(pss):
                nc.vector.tensor_scalar_mul(
                    st[:, i * WTILE : (i + 1) * WTILE], ps[:], denom[:, ft : ft + 1]
                )
            out_rows = slice(ft * P, (ft + 1) * P)
            # finer stores on the last chunk shorten the end-of-kernel tail
            npieces = 4 if j == NJ - 1 else 2
            piece = WCHUNK // npieces
            for h in range(npieces):
                out_cols = slice(j * WCHUNK + h * piece, j * WCHUNK + (h + 1) * piece)
                nc.scalar.dma_start(
                    out[out_rows, out_cols], st[:, h * piece : (h + 1) * piece]
                )

        # chunk-0 first matmul block goes ahead of everything else
        pss00 = emit_mms(0, 0, bank_major=True)

        # ---- demodulation scale: denom[f] = rsqrt(sum_{k,c} wm^2) ----
        # Emitted after the first conv block so the tiny demod matmuls do
        # not sit at the head of the in-order PE queue waiting on the DVE
        # square/sum chain.
        ssq = []
        for h in range(NH):
            sqs = []
            for k in range(K):
                sqt = wbuf.tile([P, F], f32, tag=f"sq_{h}_{k}", name=f"sq_{h}_{k}")
                nc.vector.tensor_mul(sqt[:], wm[k][:, h, :], wm[k][:, h, :])
                sqs.append(sqt)
            sst = wbuf.tile([P, F], f32, tag=f"ssq_{h}", name=f"ssq_{h}")
            nc.vector.tensor_add(sst[:], sqs[0][:], sqs[1][:])
            nc.vector.tensor_add(sst[:], sst[:], sqs[2][:])
            ssq.append(sst)
        ones = wbuf.tile([P, 1], f32, tag="ones")
        nc.vector.memset(ones[:], 1.0)
        dp = dpsum_pool.tile([P, FT], f32, tag="dpsum")
        for ft in range(FT):
            for h in range(NH):
                nc.tensor.matmul(
                    dp[:, ft : ft + 1],
                    ssq[h][:, ft * P : (ft + 1) * P],
                    ones[:],
                    start=(h == 0),
                    stop=(h == NH - 1),
                )
        denom = wbuf.tile([P, FT], f32, tag="denom")
        nc.scalar.activation(denom[:], dp[:], mybir.ActivationFunctionType.Sqrt)
        nc.vector.reciprocal(denom[:], denom[:])

        # ---- conv: chunk loads stay one chunk ahead of the matmul stream ----
        emit_copies(0, 0, pss00)
        emit_copies(0, 1, emit_mms(0, 1))
        for j in range(1, NJ):
            if j + 1 < NJ:
                emit_load(j + 1)
            for ft in range(FT):
                emit_copies(j, ft, emit_mms(j, ft))


def build_bass():
    nc = bass.Bass(name="conv1dmod")
    feat = nc.dram_tensor("feature", [C, W], mybir.dt.float32, kind="ExternalInput")
    style = nc.dram_tensor("style", [C], mybir.dt.float32, kind="ExternalInput")
    kern = nc.dram_tensor("kern", [K, C, F], mybir.dt.float32, kind="ExternalInput")
    out = nc.dram_tensor("out", [F, W], mybir.dt.bfloat16, kind="ExternalOutput")
    with tile.TileContext(nc) as tc:
        _conv1dmod_body(tc, feat, style, kern, out)
    _split_sync_waits(nc)
    return nc


_NC_CACHE = None


def kernel(feature, style, kernel):
    """Full-input entry point: shard over batch across 8 cores, run, gather."""
    global _NC_CACHE
    from concourse.bass_utils import run_bass_kernel_spmd

    if _NC_CACHE is None:
        _NC_CACHE = build_bass()
    nc = _NC_CACHE

    feature = np.ascontiguousarray(feature, dtype=np.float32)
    style = np.ascontiguousarray(style, dtype=np.float32)
    kernel = np.ascontiguousarray(kernel, dtype=np.float32)

    in_maps = [
        {"feature": feature[b], "style": style[b], "kern": kernel} for b in range(B)
    ]
    res = run_bass_kernel_spmd(nc, in_maps, core_ids=list(range(B)))
    return np.stack(
        [np.asarray(r["out"]).astype(np.float32) for r in res.results], axis=0
    )
